# revision 3
# baseline (speedup 1.0000x reference)
"""Trainium2 Bass kernel: LinearSelfAttentionTemporal (N,C,T,V)=(64,128,64,25).

Data-parallel over batch N across 8 NeuronCores (8 samples each).
Per sample the pipeline runs in the natural (C=128 partitions, L=T*V=1600
free) layout:
  - c_attn / c_proj 1x1 convs as PE matmuls contracting over C
  - cumulative sums via DVE tensor_tensor_scan along the free dim
  - softmax WITHOUT max-subtraction: logits = temp*sum_hd(wn) are in
    [0, 16*temp] (wn = wsq/denom <= 1 since denom is an inclusive cumsum),
    so exp() is safe in fp32; denom_bias adds a per-(n,h) constant to the
    logits and cancels exactly in softmax, so it is dropped.
  - per-head (8 -> 128 partition) broadcasts via 0-stride DMA replication
  - samples processed in PAIRS: elementwise ops run on (C, 2L) fused tiles
    to halve instruction count (and Pool-engine semaphore overhead); scans
    and PSUM epilogues stay per-sample on slices of the pair tiles.
Algebra: with Pi = softmax(tmp), A = cumsum(Pi)+1e-8,
  dots = cumsum(wsq*Pi)/A  =>  attn = 1/(1+dots) = A/D
  where D = 1e-8 + cumsum((wsq+1)*Pi)  (scan with data1=Pi fused add)
  y = -(w*Pi)*attn = -(w * (Pi*A)_bcast) / D   (minus folded into -Wp^T)
The reference denom clamp max(cumsum(wsq), 1e-12) is reproduced exactly by
the scan recurrence state=(wsq+state) max 1e-12 (error <= 1e-12 absolute).
"""
import os
import sys

import numpy as np

for _p in ("/opt/trn_rl_repo",):
    if _p not in sys.path and os.path.isdir(_p):
        sys.path.insert(0, _p)

import ml_dtypes
import concourse.bacc as bacc
import concourse.tile as tile
from concourse import mybir
from concourse.bass_utils import run_bass_kernel_spmd

F32 = mybir.dt.float32
BF16 = mybir.dt.bfloat16
FP16 = mybir.dt.float16
AOP = mybir.AluOpType
AFT = mybir.ActivationFunctionType

N, C, T, V = 64, 128, 64, 25
H, HD, L = 8, 16, T * V
L2 = 2 * L
NCORES = 8
NLOC = N // NCORES
G, GS = 2, NLOC // 2  # two groups of 4 samples; 2 pairs per group
# halves of L for the 2-bank psum tiles: (offset, width, sub-chunks)
HALVES = [(0, 1024, [(0, 512), (512, 512)]), (1024, 576, [(0, 512), (512, 64)])]
CHUNKS = [(0, 512), (512, 512), (1024, 512), (1536, 64)]

DEFAULT_CFG = dict(
    wn_eng="g",    # wn = wsq*rden (paired): "g" gpsimd | "d" dve
    v2_eng="g",    # v2 = w*u_b (paired)
    p2m_eng="d",   # p2m = wsq*PiB (paired)
    y_eng="d",     # y = v2*rD (paired)
)


def _act_recip(nc, out, in_):
    """Scalar-engine Reciprocal activation (HW-verified ~1.2e-5 rel err for
    normal-range inputs; inputs here are clamped >= 1e-12)."""
    ins = [nc.scalar.lower_ap(in_)]
    for arg in (0.0, 1.0, 0.0):  # bias, scale, alpha immediates
        ins.append(mybir.ImmediateValue(dtype=mybir.dt.float32, value=arg))
    return nc.scalar.add_instruction(
        mybir.InstActivation(
            name=nc.get_next_instruction_name(),
            func=mybir.ActivationFunctionType.Reciprocal,
            ins=ins,
            outs=[nc.scalar.lower_ap(out)],
        )
    )


def build_nc(cfg=None):
    """Build and compile the per-core Bass program. Returns nc."""
    cfg = {**DEFAULT_CFG, **(cfg or {})}
    from contextlib import ExitStack

    nc = bacc.Bacc("TRN2", target_bir_lowering=False, debug=False)

    x_d = nc.dram_tensor("x16", (NLOC, C, L), FP16, kind="ExternalInput").ap()
    wat_d = nc.dram_tensor("wat16", (C, C), FP16, kind="ExternalInput").ap()
    wptn_d = nc.dram_tensor("wptn_bf", (C, C), BF16, kind="ExternalInput").ap()
    iden_d = nc.dram_tensor("iden16", (C, C), FP16, kind="ExternalInput").ap()
    ba_d = nc.dram_tensor("ba", (C, 1), F32, kind="ExternalInput").ap()
    bp_d = nc.dram_tensor("bp", (C, 1), F32, kind="ExternalInput").ap()
    m64_d = nc.dram_tensor("m64bf", (C, NLOC * 32), BF16, kind="ExternalInput").ap()
    sc_d = nc.dram_tensor("sc64", (64, 1), F32, kind="ExternalInput").ap()
    out_d = nc.dram_tensor("out16", (NLOC, C, L), FP16, kind="ExternalOutput").ap()

    def eng_of(key):
        return nc.gpsimd if cfg[key] == "g" else nc.vector

    with tile.TileContext(nc) as tc, ExitStack() as ctx:
        cons = ctx.enter_context(tc.tile_pool(name="consts", bufs=1))
        xpool = ctx.enter_context(tc.tile_pool(name="xp", bufs=1))
        wpool = ctx.enter_context(tc.tile_pool(name="wp", bufs=2))
        sqpool = ctx.enter_context(tc.tile_pool(name="sqp", bufs=2))
        work = ctx.enter_context(tc.tile_pool(name="wk", bufs=2))
        soft = ctx.enter_context(tc.tile_pool(name="sf", bufs=2))
        opool = ctx.enter_context(tc.tile_pool(name="op", bufs=2))
        pspool = ctx.enter_context(tc.tile_pool(name="ps", bufs=1, space="PSUM"))

        wat_s = cons.tile([C, C], FP16)
        nc.sync.dma_start(wat_s[:], wat_d[:])
        wptn_s = cons.tile([C, C], BF16)
        nc.sync.dma_start(wptn_s[:], wptn_d[:])
        iden_s = cons.tile([C, C], FP16)
        nc.sync.dma_start(iden_s[:], iden_d[:])
        ba_s = cons.tile([C, 1], F32)
        nc.sync.dma_start(ba_s[:], ba_d[:])
        bp_s = cons.tile([C, 1], F32)
        nc.sync.dma_start(bp_s[:], bp_d[:])
        m64_s = cons.tile([C, NLOC * 32], BF16)
        nc.sync.dma_start(m64_s[:], m64_d[:])
        sc_s = cons.tile([64, 1], F32)
        nc.sync.dma_start(sc_s[:], sc_d[:])
        eps_c = cons.tile([C, 1], BF16)
        nc.gpsimd.memset(eps_c[:], 1e-12)
        epsC = eps_c[:].broadcast_to((C, L))
        z32 = cons.tile([32, 1], BF16)
        nc.gpsimd.memset(z32[:], 0.0)
        z32L = z32[:].broadcast_to((32, L))

        # all of x stays resident: (C, 8L) fp16 = 25.6KB/partition
        x_all = xpool.tile([C, NLOC * L], FP16)
        for n in range(NLOC):
            nc.sync.dma_start(x_all[:, n * L : (n + 1) * L], x_d[n])

        w_prs = {}
        wsq_prs = {}

        def a_pair(g, p, ptmp):
            w_pr = wpool.tile([C, L2], BF16, tag="w", bufs=2)
            wsq_pr = sqpool.tile([C, L2], BF16, tag="wsq", bufs=2)
            w_prs[(g, p)] = w_pr
            wsq_prs[(g, p)] = wsq_pr
            for jj in range(2):
                n = g * GS + 2 * p + jj
                o_n, o_j = n * L, jj * L
                for (ho, hw, subs) in HALVES:
                    pw = pspool.tile([C, 1024], F32, tag="pw", bufs=1)
                    for (so, sw) in subs:
                        nc.tensor.matmul(
                            pw[:, so : so + sw],
                            wat_s[:],
                            x_all[:, o_n + ho + so : o_n + ho + so + sw],
                            start=True,
                            stop=True,
                        )
                    nc.scalar.activation(
                        w_pr[:, o_j + ho : o_j + ho + hw],
                        pw[:, 0:hw],
                        AFT.Identity,
                        bias=ba_s[:],
                    )
                    nc.scalar.activation(
                        wsq_pr[:, o_j + ho : o_j + ho + hw],
                        pw[:, 0:hw],
                        AFT.Square,
                        bias=ba_s[:],
                    )
            den_pr = work.tile([C, L2], BF16, tag="den", bufs=2)
            for jj in range(2):
                sl = slice(jj * L, (jj + 1) * L)
                nc.vector.tensor_tensor_scan(
                    den_pr[:, sl], wsq_pr[:, sl], epsC, 0.0, AOP.add, AOP.max
                )
            rden_pr = work.tile([C, L2], BF16, tag="rden", bufs=2)
            _act_recip(nc, rden_pr[:], den_pr[:])
            wn_pr = work.tile([C, L2], BF16, tag="wn", bufs=2)
            eng_of("wn_eng").tensor_tensor(wn_pr[:], wsq_pr[:], rden_pr[:], AOP.mult)
            for jj in range(2):
                n = g * GS + 2 * p + jj
                j = 2 * p + jj
                for k, (o, cw) in enumerate(CHUNKS):
                    nc.tensor.matmul(
                        ptmp[0:32, k * 512 : k * 512 + cw],
                        m64_s[:, n * 32 : (n + 1) * 32],
                        wn_pr[:, jj * L + o : jj * L + o + cw],
                        start=(j == 0),
                        stop=(j == GS - 1),
                    )

        def phase_b(g, ptmp):
            r0, r1 = g * 32, (g + 1) * 32
            e_g = soft.tile([32, L], BF16, tag="e", bufs=2)
            s_g = soft.tile([32, 1], F32, tag="s", bufs=2)
            nc.scalar.activation(
                e_g[:],
                ptmp[0:32, 0:L],
                AFT.Exp,
                scale=sc_s[r0:r1, :],
                accum_out=s_g[:],
            )
            rs = soft.tile([32, 1], F32, tag="rs", bufs=2)
            nc.vector.reciprocal(rs[:], s_g[:])
            s8 = soft.tile([32, 1], F32, tag="s8", bufs=2)
            nc.vector.tensor_scalar_mul(s8[:], s_g[:], 1e-8)
            u_g = soft.tile([32, L], BF16, tag="u", bufs=2)
            nc.vector.tensor_tensor_scan(u_g[:], e_g[:], z32L, 0.0, AOP.add, AOP.add)
            nc.vector.tensor_scalar(u_g[:], u_g[:], s8[:], rs[:], AOP.add, AOP.mult)
            Pi_g = soft.tile([32, L], BF16, tag="pi", bufs=2)
            nc.vector.tensor_scalar_mul(Pi_g[:], e_g[:], rs[:])
            nc.vector.tensor_tensor(u_g[:], u_g[:], Pi_g[:], AOP.mult)
            return Pi_g, u_g

        def c_pair(g, p, Pi_g, u_g):
            PiB_pr = work.tile([C, L2], BF16, tag="pib", bufs=2)
            ub_pr = work.tile([C, L2], BF16, tag="ub", bufs=2)
            for jj in range(2):
                r = 8 * (2 * p + jj)
                sl = slice(jj * L, (jj + 1) * L)
                nc.sync.dma_start(
                    PiB_pr[:, sl],
                    Pi_g[r : r + 8, :].unsqueeze(1).broadcast_to((8, HD, L)),
                )
                nc.sync.dma_start(
                    ub_pr[:, sl],
                    u_g[r : r + 8, :].unsqueeze(1).broadcast_to((8, HD, L)),
                )
            p2m_pr = work.tile([C, L2], BF16, tag="p2m", bufs=1)
            eng_of("p2m_eng").tensor_tensor(
                p2m_pr[:], wsq_prs[(g, p)][:], PiB_pr[:], AOP.mult
            )
            D_pr = work.tile([C, L2], BF16, tag="D", bufs=1)
            for jj in range(2):
                sl = slice(jj * L, (jj + 1) * L)
                nc.vector.tensor_tensor_scan(
                    D_pr[:, sl], p2m_pr[:, sl], PiB_pr[:, sl], 1e-8, AOP.add, AOP.add
                )
            rD_pr = work.tile([C, L2], BF16, tag="rD", bufs=2)
            _act_recip(nc, rD_pr[:], D_pr[:])
            v2_pr = work.tile([C, L2], BF16, tag="v2", bufs=2)
            eng_of("v2_eng").tensor_tensor(
                v2_pr[:], w_prs[(g, p)][:], ub_pr[:], AOP.mult
            )
            y_pr = work.tile([C, L2], BF16, tag="y", bufs=2)
            eng_of("y_eng").tensor_tensor(y_pr[:], v2_pr[:], rD_pr[:], AOP.mult)

            out_pr = opool.tile([C, L2], FP16, tag="outsb", bufs=2)
            for jj in range(2):
                n = g * GS + 2 * p + jj
                o_n, o_j = n * L, jj * L
                for (ho, hw, subs) in HALVES:
                    pj = pspool.tile([C, 1024], F32, tag="pj", bufs=1)
                    for (so, sw) in subs:
                        nc.tensor.matmul(
                            pj[:, so : so + sw],
                            wptn_s[:],
                            y_pr[:, o_j + ho + so : o_j + ho + so + sw],
                            start=True,
                            stop=False,
                        )
                    for (so, sw) in subs:
                        nc.tensor.matmul(
                            pj[:, so : so + sw],
                            iden_s[:],
                            x_all[:, o_n + ho + so : o_n + ho + so + sw],
                            start=False,
                            stop=True,
                        )
                    nc.scalar.activation(
                        out_pr[:, o_j + ho : o_j + ho + hw],
                        pj[:, 0:hw],
                        AFT.Relu,
                        bias=bp_s[:],
                    )
                nc.sync.dma_start(out_d[n], out_pr[:, o_j : o_j + L])

        # software-pipelined emission: c(0) interleaves with a(1)
        ptmp0 = pspool.tile([32, 2048], F32, tag="ptmp", bufs=1)
        a_pair(0, 0, ptmp0)
        a_pair(0, 1, ptmp0)
        Pi0, u0 = phase_b(0, ptmp0)
        ptmp1 = pspool.tile([32, 2048], F32, tag="ptmp", bufs=1)
        c_pair(0, 0, Pi0, u0)
        a_pair(1, 0, ptmp1)
        c_pair(0, 1, Pi0, u0)
        a_pair(1, 1, ptmp1)
        Pi1, u1 = phase_b(1, ptmp1)
        c_pair(1, 0, Pi1, u1)
        c_pair(1, 1, Pi1, u1)

    nc.compile()
    return nc


def make_core_inputs(inputs, cfg=None):
    """Host-side prep: returns (shared_map, per_core_x_list)."""
    x = np.asarray(inputs["x"], np.float32)  # (N,C,T,V)
    Wa = np.asarray(inputs["Wa"], np.float32)
    ba = np.asarray(inputs["ba"], np.float32)
    Wp = np.asarray(inputs["Wp"], np.float32)
    bp = np.asarray(inputs["bp"], np.float32)
    temp = np.asarray(inputs["temp"], np.float32).reshape(H)
    # denom_bias adds a per-(n,h) constant to the softmax logits -> cancels.

    assert np.all(temp > 0), "kernel assumes temp > 0"
    assert temp.max() * 16.0 < 80.0, "kernel assumes exp(16*temp) fits fp32"

    xr = np.ascontiguousarray(x.reshape(N, C, L).astype(np.float16))
    wat16 = np.ascontiguousarray(Wa.T).astype(np.float16)
    wptn_bf = np.ascontiguousarray((-Wp.T)).astype(ml_dtypes.bfloat16)
    iden16 = np.eye(C, dtype=np.float16)
    m64 = np.zeros((C, NLOC * 32), np.float32)
    cc = np.arange(C)
    for n in range(NLOC):
        m64[cc, n * 32 + 8 * (n % GS) + cc // HD] = 1.0
    m64bf = m64.astype(ml_dtypes.bfloat16)
    pp = np.arange(64)
    sc64 = temp[pp % 8].reshape(64, 1).astype(np.float32)

    shared = dict(
        wat16=wat16,
        wptn_bf=wptn_bf,
        iden16=iden16,
        ba=ba.reshape(C, 1),
        bp=bp.reshape(C, 1),
        m64bf=m64bf,
        sc64=sc64,
    )
    xs = [np.ascontiguousarray(xr[i * NLOC : (i + 1) * NLOC]) for i in range(NCORES)]
    return shared, xs


_NC_CACHE = {}


def kernel(**inputs):
    cfg_key = "default"
    if cfg_key not in _NC_CACHE:
        _NC_CACHE[cfg_key] = build_nc()
    nc = _NC_CACHE[cfg_key]
    shared, xs = make_core_inputs(inputs)
    in_maps = [dict(shared, x16=xs[i]) for i in range(NCORES)]
    res = run_bass_kernel_spmd(nc, in_maps, core_ids=list(range(NCORES)))
    out = np.concatenate([res.results[i]["out16"] for i in range(NCORES)], axis=0)
    return out.reshape(N, C, T, V).astype(np.float32)


if __name__ == "__main__":
    rng = np.random.default_rng(0)
    demo = dict(
        x=rng.standard_normal((N, C, T, V)).astype(np.float32),
        Wa=rng.standard_normal((C, C)).astype(np.float32) / np.sqrt(C),
        ba=rng.standard_normal((C,)).astype(np.float32) * 0.01,
        Wp=rng.standard_normal((C, C)).astype(np.float32) / np.sqrt(C),
        bp=rng.standard_normal((C,)).astype(np.float32) * 0.01,
        temp=np.ones((H, 1), np.float32),
        denom_bias=np.zeros((H, 1, 1), np.float32),
    )
    o = kernel(**demo)
    print("out", o.shape, o.dtype, float(np.abs(o).max()))


# revision 5
# speedup vs baseline: 1.0408x; 1.0408x over previous
"""Trainium2 Bass kernel: LinearSelfAttentionTemporal (N,C,T,V)=(64,128,64,25).

Data-parallel over batch N across 8 NeuronCores (8 samples each).
Per sample the pipeline runs in the natural (C=128 partitions, L=T*V=1600
free) layout:
  - c_attn / c_proj 1x1 convs as PE matmuls contracting over C
  - cumulative sums via DVE tensor_tensor_scan along the free dim
  - softmax WITHOUT max-subtraction: logits = temp*sum_hd(wn) are in
    [0, 16*temp] (wn = wsq/denom <= 1 since denom is an inclusive cumsum),
    so exp() is safe in fp32; denom_bias adds a per-(n,h) constant to the
    logits and cancels exactly in softmax, so it is dropped.
  - per-head (8 -> 128 partition) broadcasts via 0-stride DMA replication
  - samples processed in PAIRS: elementwise ops run on (C, 2L) fused tiles
    to halve instruction count (and Pool-engine semaphore overhead); scans
    and PSUM epilogues stay per-sample on slices of the pair tiles.
Algebra: with Pi = softmax(tmp), A = cumsum(Pi)+1e-8,
  dots = cumsum(wsq*Pi)/A  =>  attn = 1/(1+dots) = A/D
  where D = 1e-8 + cumsum((wsq+1)*Pi)  (scan with data1=Pi fused add)
  y = -(w*Pi)*attn = -(w * (Pi*A)_bcast) / D   (minus folded into -Wp^T)
The reference denom clamp max(cumsum(wsq), 1e-12) is reproduced exactly by
the scan recurrence state=(wsq+state) max 1e-12 (error <= 1e-12 absolute).
"""
import os
import sys

import numpy as np

for _p in ("/opt/trn_rl_repo",):
    if _p not in sys.path and os.path.isdir(_p):
        sys.path.insert(0, _p)

import ml_dtypes
import concourse.bacc as bacc
import concourse.tile as tile
from concourse import mybir
from concourse.bass_utils import run_bass_kernel_spmd

F32 = mybir.dt.float32
BF16 = mybir.dt.bfloat16
FP16 = mybir.dt.float16
AOP = mybir.AluOpType
AFT = mybir.ActivationFunctionType

N, C, T, V = 64, 128, 64, 25
H, HD, L = 8, 16, T * V
L2 = 2 * L
NCORES = 8
NLOC = N // NCORES
G, GS = 2, NLOC // 2  # two groups of 4 samples; 2 pairs per group
# halves of L for the 2-bank psum tiles: (offset, width, sub-chunks)
HALVES = [(0, 1024, [(0, 512), (512, 512)]), (1024, 576, [(0, 512), (512, 64)])]
CHUNKS = [(0, 512), (512, 512), (1024, 512), (1536, 64)]

DEFAULT_CFG = dict(
    wn_eng="g",    # wn = wsq*rden (paired): "g" gpsimd | "d" dve
    v2_eng="g",    # v2 = w*u_b (paired)
    p2m_eng="d",   # p2m = wsq*PiB (paired)
    y_eng="d",     # y = v2*rD (paired)
)


def _act_recip(nc, out, in_):
    """Scalar-engine Reciprocal activation (HW-verified ~1.2e-5 rel err for
    normal-range inputs; inputs here are clamped >= 1e-12)."""
    ins = [nc.scalar.lower_ap(in_)]
    for arg in (0.0, 1.0, 0.0):  # bias, scale, alpha immediates
        ins.append(mybir.ImmediateValue(dtype=mybir.dt.float32, value=arg))
    return nc.scalar.add_instruction(
        mybir.InstActivation(
            name=nc.get_next_instruction_name(),
            func=mybir.ActivationFunctionType.Reciprocal,
            ins=ins,
            outs=[nc.scalar.lower_ap(out)],
        )
    )


def build_nc(cfg=None):
    """Build and compile the per-core Bass program. Returns nc."""
    cfg = {**DEFAULT_CFG, **(cfg or {})}
    from contextlib import ExitStack

    nc = bacc.Bacc("TRN2", target_bir_lowering=False, debug=False)

    x_d = nc.dram_tensor("x16", (NLOC, C, L), FP16, kind="ExternalInput").ap()
    wat_d = nc.dram_tensor("wat16", (C, C), FP16, kind="ExternalInput").ap()
    wptn_d = nc.dram_tensor("wptn_bf", (C, C), BF16, kind="ExternalInput").ap()
    iden_d = nc.dram_tensor("iden16", (C, C), FP16, kind="ExternalInput").ap()
    ba_d = nc.dram_tensor("ba", (C, 1), F32, kind="ExternalInput").ap()
    bp_d = nc.dram_tensor("bp", (C, 1), F32, kind="ExternalInput").ap()
    m64_d = nc.dram_tensor("m64bf", (C, NLOC * 32), BF16, kind="ExternalInput").ap()
    sc_d = nc.dram_tensor("sc64", (64, 1), F32, kind="ExternalInput").ap()
    out_d = nc.dram_tensor("out16", (NLOC, C, L), FP16, kind="ExternalOutput").ap()

    def eng_of(key):
        return nc.gpsimd if cfg[key] == "g" else nc.vector

    with tile.TileContext(nc) as tc, ExitStack() as ctx:
        cons = ctx.enter_context(tc.tile_pool(name="consts", bufs=1))
        xpool = ctx.enter_context(tc.tile_pool(name="xp", bufs=1))
        wpool = ctx.enter_context(tc.tile_pool(name="wp", bufs=2))
        sqpool = ctx.enter_context(tc.tile_pool(name="sqp", bufs=2))
        work = ctx.enter_context(tc.tile_pool(name="wk", bufs=2))
        soft = ctx.enter_context(tc.tile_pool(name="sf", bufs=2))
        opool = ctx.enter_context(tc.tile_pool(name="op", bufs=2))
        pspool = ctx.enter_context(tc.tile_pool(name="ps", bufs=1, space="PSUM"))

        wat_s = cons.tile([C, C], FP16)
        nc.sync.dma_start(wat_s[:], wat_d[:])
        wptn_s = cons.tile([C, C], BF16)
        nc.sync.dma_start(wptn_s[:], wptn_d[:])
        iden_s = cons.tile([C, C], FP16)
        nc.sync.dma_start(iden_s[:], iden_d[:])
        ba_s = cons.tile([C, 1], F32)
        nc.sync.dma_start(ba_s[:], ba_d[:])
        bp_s = cons.tile([C, 1], F32)
        nc.sync.dma_start(bp_s[:], bp_d[:])
        m64_s = cons.tile([C, NLOC * 32], BF16)
        nc.sync.dma_start(m64_s[:], m64_d[:])
        sc_s = cons.tile([64, 1], F32)
        nc.sync.dma_start(sc_s[:], sc_d[:])
        eps_c = cons.tile([C, 1], BF16)
        nc.gpsimd.memset(eps_c[:], 1e-12)
        epsC = eps_c[:].broadcast_to((C, L))
        z32 = cons.tile([32, 1], BF16)
        nc.gpsimd.memset(z32[:], 0.0)
        z32L = z32[:].broadcast_to((32, L))

        # all of x stays resident: (C, 8L) fp16 = 25.6KB/partition
        x_all = xpool.tile([C, NLOC * L], FP16)
        for n in range(NLOC):
            nc.sync.dma_start(x_all[:, n * L : (n + 1) * L], x_d[n])

        w_l = [None] * NLOC
        wsq_l = [None] * NLOC

        def a_one(n, ptmp):
            """c_attn + epilogues + denom pipeline + tmp-matmuls for sample n."""
            j = n % GS
            w_t = wpool.tile([C, L], BF16, tag="w", bufs=4)
            wsq_t = sqpool.tile([C, L], BF16, tag="wsq", bufs=4)
            w_l[n] = w_t
            wsq_l[n] = wsq_t
            o_n = n * L
            for (ho, hw, subs) in HALVES:
                pw = pspool.tile([C, 1024], F32, tag="pw", bufs=1)
                for (so, sw) in subs:
                    nc.tensor.matmul(
                        pw[:, so : so + sw],
                        wat_s[:],
                        x_all[:, o_n + ho + so : o_n + ho + so + sw],
                        start=True,
                        stop=True,
                    )
                nc.scalar.activation(
                    w_t[:, ho : ho + hw], pw[:, 0:hw], AFT.Identity, bias=ba_s[:]
                )
                nc.scalar.activation(
                    wsq_t[:, ho : ho + hw], pw[:, 0:hw], AFT.Square, bias=ba_s[:]
                )
            den_t = work.tile([C, L], BF16, tag="den", bufs=2)
            nc.vector.tensor_tensor_scan(
                den_t[:], wsq_t[:], epsC, 0.0, AOP.add, AOP.max
            )
            rden_t = work.tile([C, L], BF16, tag="rden", bufs=2)
            _act_recip(nc, rden_t[:], den_t[:])
            wn_t = work.tile([C, L], BF16, tag="wn", bufs=2)
            eng_of("wn_eng").tensor_tensor(wn_t[:], wsq_t[:], rden_t[:], AOP.mult)
            for k, (o, cw) in enumerate(CHUNKS):
                nc.tensor.matmul(
                    ptmp[0:32, k * 512 : k * 512 + cw],
                    m64_s[:, n * 32 : (n + 1) * 32],
                    wn_t[:, o : o + cw],
                    start=(j == 0),
                    stop=(j == GS - 1),
                )

        def phase_b(g, ptmp):
            r0, r1 = g * 32, (g + 1) * 32
            e_g = soft.tile([32, L], BF16, tag="e", bufs=2)
            s_g = soft.tile([32, 1], F32, tag="s", bufs=2)
            nc.scalar.activation(
                e_g[:],
                ptmp[0:32, 0:L],
                AFT.Exp,
                scale=sc_s[r0:r1, :],
                accum_out=s_g[:],
            )
            rs = soft.tile([32, 1], F32, tag="rs", bufs=2)
            nc.vector.reciprocal(rs[:], s_g[:])
            s8 = soft.tile([32, 1], F32, tag="s8", bufs=2)
            nc.vector.tensor_scalar_mul(s8[:], s_g[:], 1e-8)
            u_g = soft.tile([32, L], BF16, tag="u", bufs=2)
            nc.vector.tensor_tensor_scan(u_g[:], e_g[:], z32L, 0.0, AOP.add, AOP.add)
            nc.vector.tensor_scalar(u_g[:], u_g[:], s8[:], rs[:], AOP.add, AOP.mult)
            Pi_g = soft.tile([32, L], BF16, tag="pi", bufs=2)
            nc.vector.tensor_scalar_mul(Pi_g[:], e_g[:], rs[:])
            nc.vector.tensor_tensor(u_g[:], u_g[:], Pi_g[:], AOP.mult)
            return Pi_g, u_g

        PiB_l = [None] * NLOC
        ub_l = [None] * NLOC

        def c_bcast(n, Pi_g, u_g):
            """Trigger the per-head broadcasts for sample n (ready right
            after phase_b; triggers split across sync and gpsimd queues)."""
            r = 8 * (n % GS)
            PiB = work.tile([C, L], BF16, tag="pib", bufs=4)
            nc.sync.dma_start(
                PiB[:], Pi_g[r : r + 8, :].unsqueeze(1).broadcast_to((8, HD, L))
            )
            ub = work.tile([C, L], BF16, tag="ub", bufs=4)
            nc.gpsimd.dma_start(
                ub[:], u_g[r : r + 8, :].unsqueeze(1).broadcast_to((8, HD, L))
            )
            PiB_l[n] = PiB
            ub_l[n] = ub

        def c_one(n):
            """Attention-apply chain for sample n."""
            PiB, ub = PiB_l[n], ub_l[n]
            p2m = work.tile([C, L], BF16, tag="p2m", bufs=2)
            eng_of("p2m_eng").tensor_tensor(p2m[:], wsq_l[n][:], PiB[:], AOP.mult)
            D_t = work.tile([C, L], BF16, tag="D", bufs=2)
            nc.vector.tensor_tensor_scan(
                D_t[:], p2m[:], PiB[:], 1e-8, AOP.add, AOP.add
            )
            rD = work.tile([C, L], BF16, tag="rD", bufs=2)
            _act_recip(nc, rD[:], D_t[:])
            v2 = work.tile([C, L], BF16, tag="v2", bufs=2)
            eng_of("v2_eng").tensor_tensor(v2[:], w_l[n][:], ub[:], AOP.mult)
            y_t = work.tile([C, L], BF16, tag="y", bufs=2)
            eng_of("y_eng").tensor_tensor(y_t[:], v2[:], rD[:], AOP.mult)

            out_sb = opool.tile([C, L], FP16, tag="outsb", bufs=2)
            o_n = n * L
            for (ho, hw, subs) in HALVES:
                pj = pspool.tile([C, 1024], F32, tag="pj", bufs=1)
                for (so, sw) in subs:
                    nc.tensor.matmul(
                        pj[:, so : so + sw],
                        wptn_s[:],
                        y_t[:, ho + so : ho + so + sw],
                        start=True,
                        stop=False,
                    )
                for (so, sw) in subs:
                    nc.tensor.matmul(
                        pj[:, so : so + sw],
                        iden_s[:],
                        x_all[:, o_n + ho + so : o_n + ho + so + sw],
                        start=False,
                        stop=True,
                    )
                nc.scalar.activation(
                    out_sb[:, ho : ho + hw], pj[:, 0:hw], AFT.Relu, bias=bp_s[:]
                )
            nc.gpsimd.dma_start(out_d[n], out_sb[:])

        # Software-pipelined emission in data-readiness order: broadcasts fire
        # right after phase_b; each c-chain piece is interleaved between
        # independent a-chain pieces so no engine queue head-of-line blocks.
        ptmp0 = pspool.tile([32, 2048], F32, tag="ptmp", bufs=1)
        for n in range(4):
            a_one(n, ptmp0)
        Pi0, u0 = phase_b(0, ptmp0)
        for n in range(4):
            c_bcast(n, Pi0, u0)
        ptmp1 = pspool.tile([32, 2048], F32, tag="ptmp", bufs=1)
        a_one(4, ptmp1)
        c_one(0)
        a_one(5, ptmp1)
        c_one(1)
        a_one(6, ptmp1)
        c_one(2)
        a_one(7, ptmp1)
        c_one(3)
        Pi1, u1 = phase_b(1, ptmp1)
        for n in range(4, 8):
            c_bcast(n, Pi1, u1)
        for n in range(4, 8):
            c_one(n)

    nc.compile()
    return nc


def make_core_inputs(inputs, cfg=None):
    """Host-side prep: returns (shared_map, per_core_x_list)."""
    x = np.asarray(inputs["x"], np.float32)  # (N,C,T,V)
    Wa = np.asarray(inputs["Wa"], np.float32)
    ba = np.asarray(inputs["ba"], np.float32)
    Wp = np.asarray(inputs["Wp"], np.float32)
    bp = np.asarray(inputs["bp"], np.float32)
    temp = np.asarray(inputs["temp"], np.float32).reshape(H)
    # denom_bias adds a per-(n,h) constant to the softmax logits -> cancels.

    assert np.all(temp > 0), "kernel assumes temp > 0"
    assert temp.max() * 16.0 < 80.0, "kernel assumes exp(16*temp) fits fp32"

    xr = np.ascontiguousarray(x.reshape(N, C, L).astype(np.float16))
    wat16 = np.ascontiguousarray(Wa.T).astype(np.float16)
    wptn_bf = np.ascontiguousarray((-Wp.T)).astype(ml_dtypes.bfloat16)
    iden16 = np.eye(C, dtype=np.float16)
    m64 = np.zeros((C, NLOC * 32), np.float32)
    cc = np.arange(C)
    for n in range(NLOC):
        m64[cc, n * 32 + 8 * (n % GS) + cc // HD] = 1.0
    m64bf = m64.astype(ml_dtypes.bfloat16)
    pp = np.arange(64)
    sc64 = temp[pp % 8].reshape(64, 1).astype(np.float32)

    shared = dict(
        wat16=wat16,
        wptn_bf=wptn_bf,
        iden16=iden16,
        ba=ba.reshape(C, 1),
        bp=bp.reshape(C, 1),
        m64bf=m64bf,
        sc64=sc64,
    )
    xs = [np.ascontiguousarray(xr[i * NLOC : (i + 1) * NLOC]) for i in range(NCORES)]
    return shared, xs


_NC_CACHE = {}


def kernel(**inputs):
    cfg_key = "default"
    if cfg_key not in _NC_CACHE:
        _NC_CACHE[cfg_key] = build_nc()
    nc = _NC_CACHE[cfg_key]
    shared, xs = make_core_inputs(inputs)
    in_maps = [dict(shared, x16=xs[i]) for i in range(NCORES)]
    res = run_bass_kernel_spmd(nc, in_maps, core_ids=list(range(NCORES)))
    out = np.concatenate([res.results[i]["out16"] for i in range(NCORES)], axis=0)
    return out.reshape(N, C, T, V).astype(np.float32)


if __name__ == "__main__":
    rng = np.random.default_rng(0)
    demo = dict(
        x=rng.standard_normal((N, C, T, V)).astype(np.float32),
        Wa=rng.standard_normal((C, C)).astype(np.float32) / np.sqrt(C),
        ba=rng.standard_normal((C,)).astype(np.float32) * 0.01,
        Wp=rng.standard_normal((C, C)).astype(np.float32) / np.sqrt(C),
        bp=rng.standard_normal((C,)).astype(np.float32) * 0.01,
        temp=np.ones((H, 1), np.float32),
        denom_bias=np.zeros((H, 1, 1), np.float32),
    )
    o = kernel(**demo)
    print("out", o.shape, o.dtype, float(np.abs(o).max()))
